# revision 6
# baseline (speedup 1.0000x reference)
"""Trainium2 Bass kernel for nn_CausalPerformer (causal linear attention).

Self-contained: kernel(**inputs) -> np.ndarray.

Strategy (8 NeuronCores, SPMD over sequence rows):
  - Flatten (B,S) -> 8192 rows; core i owns rows [i*1024, (i+1)*1024).
  - Host prep: transpose+cast activations to bf16; fuse omega into W_q/W_k
    (x = q @ (omega@W_q).T), so only V and O need full 1024x1024 projections.
  - q' normalization cancels in num/den -> skipped. k' normalization folds
    into a scaled V matrix (Vtil = [vh*recip | recip]).
  - Launch 1 (kernel1): k features, V projection, per-chunk state deltas.
  - Host: tiny exclusive prefix-sum of the (7x65) chunk states per (b,h).
  - Launch 2 (kernel2): q features, masked intra-chunk attention + state
    term, divide, output projection.
"""

import numpy as np
import ml_dtypes

import concourse.bacc as bacc
import concourse.mybir as mybir
from concourse import tile
from concourse.bass_utils import run_bass_kernel_spmd

BF16 = mybir.dt.bfloat16
F32 = mybir.dt.float32
NPBF16 = ml_dtypes.bfloat16

B, S, D = 2, 4096, 1024
H, DK, F = 16, 64, 7
NC = 8
RPC = B * S // NC          # 1024 rows per core
CH = 128                   # chunk length
NCH = RPC // CH            # 8 chunks per core
NDT = D // 128             # 8 D-chunks
EPS = 1e-6
ACT_EXP = mybir.ActivationFunctionType.Exp
ACT_SQ = mybir.ActivationFunctionType.Square

_cache = {}


def _bacc():
    return bacc.Bacc("TRN2", target_bir_lowering=False, debug=False, num_devices=NC)


def build_kernel1():
    nc = _bacc()
    kT = nc.dram_tensor("kT", [D, RPC], BF16, kind="ExternalInput")
    vT = nc.dram_tensor("vT", [D, RPC], BF16, kind="ExternalInput")
    wfk_pad = nc.dram_tensor("wfk_pad", [4, 128, NDT * 128], BF16, kind="ExternalInput")
    wfk_nat = nc.dram_tensor("wfk_nat", [128, NDT * 112], BF16, kind="ExternalInput")
    wvT = nc.dram_tensor("wvT", [D, D], BF16, kind="ExternalInput")
    kpt = nc.dram_tensor("kpt", [4, 128, RPC], BF16, kind="ExternalOutput")
    vtil = nc.dram_tensor("vtil", [NCH, 128, H * 65], BF16, kind="ExternalOutput")
    st = nc.dram_tensor("st", [NCH, 128, 4 * 65], F32, kind="ExternalOutput")

    with tile.TileContext(nc) as tc:
        with (
            tc.tile_pool(name="pers", bufs=1) as pers,
            tc.tile_pool(name="work", bufs=2) as work,
            tc.tile_pool(name="ps", bufs=2, space="PSUM") as ps,
        ):
            kt_sb, vt_sb, wv_sb = [], [], []
            for c in range(NDT):
                t1 = pers.tile([128, RPC], BF16, tag=f"kt{c}")
                nc.sync.dma_start(out=t1[:], in_=kT.ap()[c * 128:(c + 1) * 128, :])
                kt_sb.append(t1)
                t2 = pers.tile([128, RPC], BF16, tag=f"vt{c}")
                nc.sync.dma_start(out=t2[:], in_=vT.ap()[c * 128:(c + 1) * 128, :])
                vt_sb.append(t2)
                t3 = pers.tile([128, D], BF16, tag=f"wv{c}")
                nc.sync.dma_start(out=t3[:], in_=wvT.ap()[c * 128:(c + 1) * 128, :])
                wv_sb.append(t3)
            wfkp_sb = []
            for j in range(4):
                t = pers.tile([128, NDT * 128], BF16, tag=f"wfkp{j}")
                nc.sync.dma_start(out=t[:], in_=wfk_pad.ap()[j])
                wfkp_sb.append(t)
            wfkn_sb = pers.tile([128, NDT * 112], BF16, tag="wfkn")
            nc.sync.dma_start(out=wfkn_sb[:], in_=wfk_nat.ap()[:, :])

            # ---- phase A: transposed k' feature tiles (4 heads x 32 rows each) ----
            for j in range(4):
                for hf in range(2):
                    pf = ps.tile([128, 512], F32, tag="kft", bufs=2)
                    for c in range(NDT):
                        nc.tensor.matmul(
                            pf[:],
                            wfkp_sb[j][:, c * 128:(c + 1) * 128],
                            kt_sb[c][:, hf * 512:(hf + 1) * 512],
                            start=(c == 0), stop=(c == NDT - 1),
                        )
                    sq = work.tile([128, 512], F32, tag="sq")
                    nc.scalar.activation(sq[:], pf[:], ACT_SQ)
                    kpe = work.tile([128, 512], BF16, tag="kpe")
                    nc.scalar.activation(kpe[:], sq[:], ACT_EXP, scale=-0.5)
                    nc.sync.dma_start(
                        out=kpt.ap()[j][:, hf * 512:(hf + 1) * 512], in_=kpe[:]
                    )

            # ---- phase B: per chunk: k'nat, recip, vh, Vtil, state deltas ----
            for t in range(NCH):
                tsl = slice(t * 128, (t + 1) * 128)
                pkn = ps.tile([128, 112], F32, tag="kn", bufs=2)
                for c in range(NDT):
                    nc.tensor.matmul(
                        pkn[:],
                        kt_sb[c][:, tsl],
                        wfkn_sb[:, c * 112:(c + 1) * 112],
                        start=(c == 0), stop=(c == NDT - 1),
                    )
                sqn = work.tile([128, 112], F32, tag="sqn")
                nc.scalar.activation(sqn[:], pkn[:], ACT_SQ)
                kexp = work.tile([128, 112], F32, tag="kexp")
                nc.scalar.activation(kexp[:], sqn[:], ACT_EXP, scale=-0.5)
                knb = work.tile([128, 112], BF16, tag="knb")
                nc.vector.tensor_copy(knb[:], kexp[:])
                rc = work.tile([128, 16], F32, tag="rc")
                nc.vector.reduce_sum(
                    rc[:], kexp[:].rearrange("p (h f) -> p h f", f=F),
                    axis=mybir.AxisListType.X,
                )
                nc.vector.tensor_scalar_add(rc[:], rc[:], EPS)
                nc.vector.reciprocal(rc[:], rc[:])

                vts = work.tile([128, H * 65], BF16, tag="vts")
                for hf in range(2):
                    pv = ps.tile([128, 512], F32, tag="vh", bufs=2)
                    for c in range(NDT):
                        nc.tensor.matmul(
                            pv[:],
                            vt_sb[c][:, tsl],
                            wv_sb[c][:, hf * 512:(hf + 1) * 512],
                            start=(c == 0), stop=(c == NDT - 1),
                        )
                    for hh in range(8):
                        h = hf * 8 + hh
                        nc.vector.tensor_scalar_mul(
                            vts[:, h * 65:h * 65 + 64],
                            pv[:, hh * 64:(hh + 1) * 64],
                            rc[:, h:h + 1],
                        )
                nc.vector.tensor_copy(
                    vts[:].rearrange("p (h n) -> p h n", n=65)[:, :, 64:65],
                    rc[:].rearrange("p (h o) -> p h o", o=1),
                )
                nc.sync.dma_start(out=vtil.ap()[t], in_=vts[:])

                sts = work.tile([128, 4 * 65], F32, tag="sts")
                for j in range(4):
                    pst = ps.tile([128, 65], F32, tag="st", bufs=2)
                    nc.vector.memset(pst[:], 0.0)
                    for g in range(4):
                        h = 4 * j + g
                        nc.tensor.matmul(
                            pst[32 * g:32 * g + 7, :],
                            knb[:, 7 * h:7 * h + 7],
                            vts[:, 65 * h:65 * h + 65],
                            start=True, stop=True,
                            tile_position=(0, 32 * g),
                        )
                    nc.vector.tensor_copy(sts[:, j * 65:(j + 1) * 65], pst[:])
                nc.sync.dma_start(out=st.ap()[t], in_=sts[:])

    nc.compile()
    return nc


def build_kernel2():
    nc = _bacc()
    qT = nc.dram_tensor("qT", [D, RPC], BF16, kind="ExternalInput")
    wfq_pad = nc.dram_tensor("wfq_pad", [4, 128, NDT * 128], BF16, kind="ExternalInput")
    kpt = nc.dram_tensor("kpt", [4, 128, RPC], BF16, kind="ExternalInput")
    vtil = nc.dram_tensor("vtil", [NCH, 128, H * 65], BF16, kind="ExternalInput")
    stin = nc.dram_tensor("stin", [128, NCH * 4 * 65], BF16, kind="ExternalInput")
    woT = nc.dram_tensor("woT", [D, D], BF16, kind="ExternalInput")
    consts = nc.dram_tensor("consts", [128, 256], BF16, kind="ExternalInput")
    o = nc.dram_tensor("o", [RPC, D], F32, kind="ExternalOutput")

    with tile.TileContext(nc) as tc:
        with (
            tc.tile_pool(name="pers", bufs=1) as pers,
            tc.tile_pool(name="work", bufs=2) as work,
        ):
            qt_sb, wo_sb = [], []
            for c in range(NDT):
                t1 = pers.tile([128, RPC], BF16, tag=f"qt{c}")
                nc.sync.dma_start(out=t1[:], in_=qT.ap()[c * 128:(c + 1) * 128, :])
                qt_sb.append(t1)
                t2 = pers.tile([128, D], BF16, tag=f"wo{c}")
                nc.sync.dma_start(out=t2[:], in_=woT.ap()[c * 128:(c + 1) * 128, :])
                wo_sb.append(t2)
            wfqp_sb, kpt_sb, vts_sb = [], [], []
            for j in range(4):
                t = pers.tile([128, NDT * 128], BF16, tag=f"wfqp{j}")
                nc.sync.dma_start(out=t[:], in_=wfq_pad.ap()[j])
                wfqp_sb.append(t)
                t = pers.tile([128, RPC], BF16, tag=f"kpt{j}")
                nc.sync.dma_start(out=t[:], in_=kpt.ap()[j])
                kpt_sb.append(t)
            for t in range(NCH):
                tt = pers.tile([128, H * 65], BF16, tag=f"vts{t}")
                nc.sync.dma_start(out=tt[:], in_=vtil.ap()[t])
                vts_sb.append(tt)
            st_sb = pers.tile([128, NCH * 4 * 65], BF16, tag="st")
            nc.sync.dma_start(out=st_sb[:], in_=stin.ap()[:, :])
            cst = pers.tile([128, 256], BF16, tag="cst")
            nc.sync.dma_start(out=cst[:], in_=consts.ap()[:, :])
            mask_tri = cst[:, 0:128]
            ident = cst[:, 128:256]

            # ---- q' feature tiles ----
            qpt_sb = []
            with tc.tile_pool(name="psq", bufs=2, space="PSUM") as psq:
                for j in range(4):
                    qp = pers.tile([128, RPC], BF16, tag=f"qpt{j}")
                    for hf in range(2):
                        pf = psq.tile([128, 512], F32, tag="qft")
                        for c in range(NDT):
                            nc.tensor.matmul(
                                pf[:],
                                wfqp_sb[j][:, c * 128:(c + 1) * 128],
                                qt_sb[c][:, hf * 512:(hf + 1) * 512],
                                start=(c == 0), stop=(c == NDT - 1),
                            )
                        sq = work.tile([128, 512], F32, tag="sq")
                        nc.scalar.activation(sq[:], pf[:], ACT_SQ)
                        nc.scalar.activation(
                            qp[:, hf * 512:(hf + 1) * 512], sq[:], ACT_EXP, scale=-0.5
                        )
                    qpt_sb.append(qp)

            # ---- attention chunks ----
            with tc.tile_pool(name="psm", bufs=1, space="PSUM") as psm:
                for t in range(NCH):
                    tsl = slice(t * 128, (t + 1) * 128)
                    atm = []
                    for h in range(H):
                        j, g = divmod(h, 4)
                        pat = psm.tile([128, 128], F32, tag="at", bufs=2)
                        nc.tensor.matmul(
                            pat[:],
                            kpt_sb[j][32 * g:32 * g + 7, tsl],
                            qpt_sb[j][32 * g:32 * g + 7, tsl],
                            start=True, stop=True,
                            tile_position=(32 * g, 0),
                        )
                        am = work.tile([128, 128], BF16, tag="atm", bufs=4)
                        nc.vector.tensor_mul(am[:], pat[:], mask_tri)
                        atm.append(am)
                    # numerators: 3 psum tiles pack 7+7+2 heads
                    pn = [
                        psm.tile([128, 7 * 65], F32, tag="num", bufs=3, name="pn0"),
                        psm.tile([128, 7 * 65], F32, tag="num", bufs=3, name="pn1"),
                        psm.tile([128, 2 * 65], F32, tag="num", bufs=3, name="pn2"),
                    ]
                    for h in range(H):
                        j, g = divmod(h, 4)
                        bi, hh = divmod(h, 7)
                        out_ap = pn[bi][:, hh * 65:(hh + 1) * 65]
                        nc.tensor.matmul(
                            out_ap, atm[h][:], vts_sb[t][:, h * 65:(h + 1) * 65],
                            start=True, stop=False,
                        )
                        nc.tensor.matmul(
                            out_ap,
                            qpt_sb[j][32 * g:32 * g + 7, tsl],
                            st_sb[32 * g:32 * g + 7, (t * 4 + j) * 65:(t * 4 + j + 1) * 65],
                            start=False, stop=True,
                            tile_position=(32 * g, 0),
                        )
                    den = work.tile([128, 16], F32, tag="den", bufs=2)
                    for bi, cnt in ((0, 7), (1, 7), (2, 2)):
                        nc.vector.tensor_scalar_add(
                            den[:, bi * 7:bi * 7 + cnt].rearrange("p (h o) -> p h o", o=1),
                            pn[bi][:].rearrange("p (h n) -> p h n", n=65)[:, :, 64:65],
                            EPS,
                        )
                    nc.vector.reciprocal(den[:], den[:])
                    # divide (ACT, per head) then PE-transpose head pairs
                    ohT = []
                    for p in range(8):
                        oh = work.tile([128, 128], BF16, tag="oh", bufs=3)
                        for hp in range(2):
                            h = 2 * p + hp
                            bi, hh = divmod(h, 7)
                            nc.scalar.mul(
                                oh[:, hp * 64:(hp + 1) * 64],
                                pn[bi][:, hh * 65:hh * 65 + 64],
                                den[:, h:h + 1],
                            )
                        ptr = psm.tile([128, 128], BF16, tag="tr", bufs=1)
                        nc.tensor.transpose(ptr[:], oh[:], ident)
                        otr = work.tile([128, 128], BF16, tag="otr", bufs=8)
                        nc.vector.tensor_copy(otr[:], ptr[:])
                        ohT.append(otr)
                    # output projection
                    osb = work.tile([128, D], F32, tag="osb", bufs=2)
                    for hf in range(2):
                        po = psm.tile([128, 512], F32, tag="po", bufs=2)
                        for p in range(8):
                            nc.tensor.matmul(
                                po[:],
                                ohT[p][:],
                                wo_sb[p][:, hf * 512:(hf + 1) * 512],
                                start=(p == 0), stop=(p == 7),
                            )
                        nc.vector.tensor_copy(osb[:, hf * 512:(hf + 1) * 512], po[:])
                    nc.sync.dma_start(out=o.ap()[tsl, :], in_=osb[:])

    nc.compile()
    return nc


def _host_prep(q, k, v, w_q, w_k, w_v, w_o, omega):
    """Host-side input marshaling: transposes, casts, weight fusion."""
    Wfq = np.einsum("fd,hdD->hfD", omega, w_q.reshape(H, DK, D)).reshape(H * F, D)
    Wfk = np.einsum("fd,hdD->hfD", omega, w_k.reshape(H, DK, D)).reshape(H * F, D)

    def pad_tiles(Wf):
        # [4, 128p(D-in-chunk), 8c*128m] lhsT tiles; col 32g+f = Wf[(4j+g)*7+f]
        out = np.zeros((4, 128, NDT * 128), np.float32)
        for j in range(4):
            wt = np.zeros((D, 128), np.float32)
            for g in range(4):
                wt[:, 32 * g:32 * g + 7] = Wf[(4 * j + g) * 7:(4 * j + g) * 7 + 7].T
            out[j] = wt.reshape(NDT, 128, 128).transpose(1, 0, 2).reshape(128, NDT * 128)
        return out.astype(NPBF16)

    wfq_pad = pad_tiles(Wfq)
    wfk_pad = pad_tiles(Wfk)
    # k'nat rhs: [128p, 8c*112]; chunk c cols = Wfk.T[c*128:(c+1)*128, :]
    wfk_nat = (
        Wfk.T.reshape(NDT, 128, H * F).transpose(1, 0, 2).reshape(128, NDT * H * F)
    ).astype(NPBF16)
    wvT = np.ascontiguousarray(w_v.T).astype(NPBF16)
    woT = np.ascontiguousarray(w_o.T).astype(NPBF16)

    qf = q.reshape(B * S, D)
    kf = k.reshape(B * S, D)
    vf = v.reshape(B * S, D)
    qT, kT, vT = [], [], []
    for i in range(NC):
        rows = slice(i * RPC, (i + 1) * RPC)
        qT.append(np.ascontiguousarray(qf[rows].T).astype(NPBF16))
        kT.append(np.ascontiguousarray(kf[rows].T).astype(NPBF16))
        vT.append(np.ascontiguousarray(vf[rows].T).astype(NPBF16))

    mask_tri = (np.arange(CH)[:, None] <= np.arange(CH)[None, :]).astype(NPBF16)
    consts = np.zeros((128, 256), NPBF16)
    consts[:, 0:128] = mask_tri
    consts[:, 128:256] = np.eye(128, dtype=NPBF16)
    return dict(
        wfq_pad=wfq_pad, wfk_pad=wfk_pad, wfk_nat=wfk_nat, wvT=wvT, woT=woT,
        qT=qT, kT=kT, vT=vT, consts=consts,
    )


def _host_prefix(st_cores):
    """Exclusive prefix over chunk deltas -> per-core/chunk input states (bf16).

    st_cores[i]: [NCH, 128, 4*65] f32; rows 32g+f of block j = head 4j+g.
    Returns stin[i]: [128, NCH*4*65] bf16.
    """
    # delta[b, c, p, j, n] with c = global chunk in b (32)
    st = np.stack(st_cores).reshape(2, 4, NCH, 128, 4, 65)
    st = st.reshape(2, 32, 128, 4, 65)
    pref = np.cumsum(st, axis=1, dtype=np.float64) - st       # exclusive
    pref = pref.astype(np.float32).reshape(2, 4, NCH, 128, 4, 65).reshape(
        NC, NCH, 128, 4, 65)
    stin = pref.transpose(0, 2, 1, 3, 4).reshape(NC, 128, NCH * 4 * 65)
    # zero the padding rows (32g+7 .. 32g+31) for cleanliness
    stv = stin.reshape(NC, 4, 32, NCH * 4 * 65)
    stv[:, :, 7:, :] = 0.0
    return [np.ascontiguousarray(stin[i]).astype(NPBF16) for i in range(NC)]


def kernel(q, k, v, w_q, w_k, w_v, w_o, omega):
    q = np.asarray(q, np.float32)
    k = np.asarray(k, np.float32)
    v = np.asarray(v, np.float32)
    w_q = np.asarray(w_q, np.float32)
    w_k = np.asarray(w_k, np.float32)
    w_v = np.asarray(w_v, np.float32)
    w_o = np.asarray(w_o, np.float32)
    omega = np.asarray(omega, np.float32)

    hp = _host_prep(q, k, v, w_q, w_k, w_v, w_o, omega)

    if "nc1" not in _cache:
        _cache["nc1"] = build_kernel1()
    nc1 = _cache["nc1"]
    in1 = [
        dict(kT=hp["kT"][i], vT=hp["vT"][i], wfk_pad=hp["wfk_pad"],
             wfk_nat=hp["wfk_nat"], wvT=hp["wvT"])
        for i in range(NC)
    ]
    r1 = run_bass_kernel_spmd(nc1, in1, core_ids=list(range(NC)))
    stin = _host_prefix([r1.results[i]["st"] for i in range(NC)])

    if "nc2" not in _cache:
        _cache["nc2"] = build_kernel2()
    nc2 = _cache["nc2"]
    in2 = [
        dict(qT=hp["qT"][i], wfq_pad=hp["wfq_pad"], kpt=r1.results[i]["kpt"],
             vtil=r1.results[i]["vtil"], stin=stin[i], woT=hp["woT"],
             consts=hp["consts"])
        for i in range(NC)
    ]
    r2 = run_bass_kernel_spmd(nc2, in2, core_ids=list(range(NC)))
    out = np.concatenate([r2.results[i]["o"] for i in range(NC)], axis=0)
    return out.reshape(B, S, D)
